# revision 1
# baseline (speedup 1.0000x reference)
"""MixConv depthwise conv (3x3/5x5/7x7 over 64-channel groups) on 8 NeuronCores.

Per core: 24 channels (8 of each kernel size). The 5x5/7x7 channels (and any
k=3 channels not offloaded) run as banded-Toeplitz matmuls on the TensorEngine:
a kxk depthwise conv = sum over dx of a 1D conv along H (a banded [H, H]
Toeplitz matmul contracting over H=112 partitions, folding all k dy-taps),
with W-shifts as free-dim offsets into a padded SBUF tile and dx-passes
accumulating in PSUM (pass-major across all 8 banks).

7 of the 8 3x3 channels are offloaded to the otherwise-idle VectorEngine:
layout [128 partitions = (image, 28-row H-block)], free dim = halo'd 30x114
patch, so all 9 taps are free-dim offsets (engine APs must start at partition
0, so H-shifts cannot be partition offsets). Each tap = tensor_scalar_mul +
tensor_add, ping-ponging two bf16 accumulators; a column-shifted copy of x
keeps every read 4-byte aligned for the DVE 2x/4x packed modes. PSUM drains
split 1 bank/channel to DVE (cheap single-bank copies, ~0.5us) and 7 to the
ScalarEngine (flat ~1.5us/op), balancing all three engine spans just under
the PE's ~160us matmul stream.

Everything rides in bf16 (PSUM accumulates fp32); HBM traffic halves and the
TensorEngine matmul stream is the binding resource. Measured ~175-200us vs
the 840us fp32 baseline.
"""

import numpy as np
import ml_dtypes

import concourse.bacc as bacc
import concourse.mybir as mybir
import concourse.tile as tile
from concourse.bass_utils import run_bass_kernel_spmd

BF16 = ml_dtypes.bfloat16

# Problem constants (hardcoded per contract)
N_IMGS = 32
H = W = 112
GROUP_KS = (3, 5, 7)
GROUP_SIZE = 64          # channels per group
N_CORES = 8
CH_PER_GROUP_PER_CORE = GROUP_SIZE // N_CORES   # 8
CH_PER_CORE = CH_PER_GROUP_PER_CORE * len(GROUP_KS)  # 24

RW = W + 6               # per-image region width in the padded tile (max pad=3)
DATA_OFF = 3             # data cols at [3, 115) of each region
XCOLS = N_IMGS * RW + 8  # +8 slack for last-chunk matmul over-read
OCOLS = N_IMGS * W
N_MM = 4 * RW            # 472 — matmul free dim (4 images/chunk)
N_CHUNK = N_IMGS // 4    # 8 chunks = 8 PSUM banks

# DVE-offload layout: partitions = (image, H-block of 28 rows), free = patch
HB = 28
RH = HB + 2              # stored rows per partition (1 halo row each side)
RWP = W + 2              # stored cols per row (1 pad col each side)
XDF = RH * RWP + 4       # 3424 (+4 slack for shifted over-read)
ODF = HB * W             # 3136 out elems per partition
TAPS3 = [(dy, dx) for dy in (-1, 0, 1) for dx in (-1, 0, 1)]

# Tuning (fixed by config sweeps on hardware)
N3 = 7                   # 3x3 channels offloaded to the Vector engine
TAP_MODE = "vs"          # conv taps as tensor_scalar_mul + tensor_add, all DVE
DSPLIT = 1.0             # PSUM banks per channel drained on DVE (rest on ACT)

MM_MODE = f"bf16 n3={N3} {TAP_MODE}"  # informational (test.py prints it)

# Per-core channel order: interleave 7,5,3 so early big-k channels cover the
# DMA prefetch of later ones.
CORE_KS = [7, 5, 3] * CH_PER_GROUP_PER_CORE
DVE_POS = [i for i, k in enumerate(CORE_KS) if k == 3][:N3]
PE_POS = [i for i in range(CH_PER_CORE) if i not in DVE_POS]
KS_PE = [CORE_KS[i] for i in PE_POS]
TOFF = np.cumsum([0] + KS_PE).tolist()    # tmat slice offset per PE channel
N_TMAT = TOFF[-1]
N_PE = len(PE_POS)

_BASS_CACHE = {}


def _build_bass(reps=1):
    bf = mybir.dt.bfloat16
    nc = bacc.Bacc("TRN2", target_bir_lowering=False, debug=False)
    xp_d = nc.dram_tensor("xp", [N_PE, H, XCOLS], bf, kind="ExternalInput")
    t_d = nc.dram_tensor("tmat", [N_TMAT * H, H], bf, kind="ExternalInput")
    y_d = nc.dram_tensor("y", [N_PE, H, OCOLS], bf, kind="ExternalOutput")
    if N3:
        xd_d = nc.dram_tensor("xd", [N3, 128, XDF], bf, kind="ExternalInput")
        w_d = nc.dram_tensor("wdve", [128, N3 * 9], mybir.dt.float32, kind="ExternalInput")
        y2_d = nc.dram_tensor("y2", [N3, 128, ODF], bf, kind="ExternalOutput")

    with tile.TileContext(nc) as tc:
        with (
            tc.tile_pool(name="xpool", bufs=4) as xpool,
            tc.tile_pool(name="tpool", bufs=1) as tpool,
            tc.tile_pool(name="opool", bufs=3) as opool,
            tc.tile_pool(name="dpool", bufs=2) as dpool,
            tc.tile_pool(name="spool", bufs=3) as spool,
            tc.tile_pool(name="pspool", bufs=8, space="PSUM") as pspool,
        ):
            # Toeplitz bank resident in SBUF, one per-channel slice DMA each.
            t_t = tpool.tile([H, N_TMAT * H], bf, tag="t", name="tmat_sb")
            for ci in range(N_PE):
                k = KS_PE[ci]
                nc.sync.dma_start(
                    t_t[:, TOFF[ci] * H : (TOFF[ci] + k) * H],
                    t_d[TOFF[ci] * H : (TOFF[ci] + k) * H].rearrange(
                        "(p d) m -> p (d m)", d=k
                    ),
                )
            if N3:
                w_t = tpool.tile([128, N3 * 9], mybir.dt.float32, tag="w", name="wdve_sb")
                nc.sync.dma_start(w_t[:, :], w_d[:, :])

            def pe_channel(ci):
                k = KS_PE[ci]
                pad = (k - 1) // 2
                x_t = xpool.tile([H, XCOLS], bf, tag="x", name=f"x{ci}")
                nc.sync.dma_start(x_t[:, :], xp_d[ci])
                out_t = opool.tile([H, OCOLS], bf, tag="o", name=f"o{ci}")
                # 8 single-bank PSUM tiles: cross-bank PSUM reads cost a flat
                # ~2.9us, single-bank DVE drains ~0.5us.
                pts = [
                    pspool.tile([H, N_MM], mybir.dt.float32, tag="ps",
                                name=f"ps{ci}_{b}")
                    for b in range(N_CHUNK)
                ]
                # pass-major: one lhsT load per dx, all 8 banks stream under it
                for dx in range(k):
                    off = dx - pad + DATA_OFF
                    lhsT = t_t[:, (TOFF[ci] + dx) * H : (TOFF[ci] + dx + 1) * H]
                    for b in range(N_CHUNK):
                        base = 4 * b * RW
                        nc.tensor.matmul(
                            pts[b],
                            lhsT=lhsT,
                            rhs=x_t[:, base + off : base + off + N_MM],
                            start=(dx == 0),
                            stop=(dx == k - 1),
                        )
                ov = out_t.rearrange("p (i w) -> p i w", i=N_IMGS)
                for b in range(N_CHUNK):
                    img0 = 4 * b
                    src = pts[b].rearrange("p (i r) -> p i r", i=4)[:, :, :W]
                    dst = ov[:, img0 : img0 + 4, :]
                    # Split drains: DVE single-bank copies are ~0.5us but
                    # DVE carries the offloaded conv chains; ACT takes the rest
                    # at its flat ~1.5-2.6us per-op cost.
                    if b < int(DSPLIT) + (1 if (DSPLIT % 1) and ci % 2 else 0):
                        nc.vector.tensor_copy(out=dst, in_=src)
                    else:
                        nc.scalar.copy(dst, src)
                    if b == N_CHUNK // 2 - 1:
                        nc.sync.dma_start(
                            y_d[ci][:, : OCOLS // 2], out_t[:, : OCOLS // 2]
                        )
                nc.sync.dma_start(y_d[ci][:, OCOLS // 2 :], out_t[:, OCOLS // 2 :])

            def dve_channel(di):
                x_e = dpool.tile([128, XDF], bf, tag="xe", name=f"xe{di}")
                nc.sync.dma_start(x_e[:, :], xd_d[di])
                x_o = dpool.tile([128, XDF], bf, tag="xo", name=f"xo{di}")
                nc.sync.dma_start(x_o[:, : XDF - 1], xd_d[di][:, 1:XDF])
                accs = [
                    spool.tile([128, ODF], bf, tag=f"a{j}", name=f"acc{j}_{di}")
                    for j in range(2)
                ]

                def tap_ap(dy, dx):
                    off = (1 + dy) * RWP + (1 + dx)
                    src, o = (x_e, off) if off % 2 == 0 else (x_o, off - 1)
                    return src[:, o : o + HB * RWP].rearrange(
                        "p (r c) -> p r c", r=HB
                    )[:, :, :W]

                def wap(t):
                    return w_t[:, di * 9 + t : di * 9 + t + 1]

                def scale_to(dst, t):
                    dy, dx = TAPS3[t]
                    # "mix": one scale per channel on DVE, rest on ACT
                    on_act = TAP_MODE == "as" or (TAP_MODE == "mix" and t != 0)
                    if on_act:
                        nc.scalar.activation(
                            dst, tap_ap(dy, dx),
                            mybir.ActivationFunctionType.Copy, scale=wap(t),
                        )
                    else:
                        nc.vector.tensor_scalar_mul(dst, tap_ap(dy, dx), wap(t))

                a3 = [a.rearrange("p (r c) -> p r c", r=HB) for a in accs]
                scale_to(a3[0], 0)
                cur = 0
                for t in range(1, 9):
                    nxt = 1 - cur
                    if TAP_MODE == "stt":
                        dy, dx = TAPS3[t]
                        nc.vector.scalar_tensor_tensor(
                            out=a3[nxt], in0=tap_ap(dy, dx), scalar=wap(t),
                            in1=a3[cur],
                            op0=mybir.AluOpType.mult, op1=mybir.AluOpType.add,
                        )
                    else:
                        s_t = spool.tile([128, ODF], bf, tag="s",
                                         name=f"s{di}_{t}")
                        s3 = s_t.rearrange("p (r c) -> p r c", r=HB)
                        scale_to(s3, t)
                        # "mix": spill 3 of the 8 adds per channel to GPSIMD
                        if TAP_MODE == "mix" and t in (2, 5, 8):
                            nc.gpsimd.tensor_add(a3[nxt], a3[cur], s3)
                        else:
                            nc.vector.tensor_add(a3[nxt], a3[cur], s3)
                    cur = nxt
                nc.sync.dma_start(y2_d[di], accs[cur][:, :])

            def body():
                pe_i = dve_i = 0
                for pos in range(CH_PER_CORE):
                    if pos in DVE_POS:
                        dve_channel(dve_i)
                        dve_i += 1
                    else:
                        pe_channel(pe_i)
                        pe_i += 1

            if reps == 1:
                body()
            else:
                with tc.For_i(0, reps, 1):
                    body()
    nc.compile()
    return nc


def _get_bass(reps=1):
    if reps not in _BASS_CACHE:
        _BASS_CACHE[reps] = _build_bass(reps)
    return _BASS_CACHE[reps]


def _build_toeplitz(w, k):
    """w: [C, 1, k, k] -> T: [C, k, H, H], T[c,dx,hin,hout] = w[c,0,hin-hout+pad,dx]."""
    pad = (k - 1) // 2
    C = w.shape[0]
    T = np.zeros((C, k, H, H), np.float32)
    for dy in range(k):
        off = pad - dy  # hout = hin + off
        hin = np.arange(max(0, -off), H - max(0, off))
        T[:, :, hin, hin + off] = w[:, 0, dy, :][:, :, None]
    return T


def _core_channels(core):
    """Global channel ids in this core's processing order (7,5,3 interleave)."""
    out = []
    for j in range(CH_PER_GROUP_PER_CORE):
        for g in (2, 1, 0):  # k=7, 5, 3 groups
            out.append(g * GROUP_SIZE + core * CH_PER_GROUP_PER_CORE + j)
    return out


def _prepare_in_maps(x, w3, w5, w7):
    x = np.asarray(x, dtype=np.float32).astype(BF16)
    ws = {3: np.asarray(w3, np.float32), 5: np.asarray(w5, np.float32),
          7: np.asarray(w7, np.float32)}
    Ts = {k: _build_toeplitz(ws[k], k) for k in (5, 7) if True}
    Ts[3] = _build_toeplitz(ws[3], 3)

    in_maps = []
    for core in range(N_CORES):
        chs = _core_channels(core)
        pe_chs = [chs[i] for i in PE_POS]
        dve_chs = [chs[i] for i in DVE_POS]

        # staged x (PE): [N_PE, H, XCOLS], data at [i*RW+3, i*RW+115) per image
        xp = np.zeros((N_PE, H, XCOLS), BF16)
        xv = xp[:, :, : N_IMGS * RW].reshape(N_PE, H, N_IMGS, RW)
        xv[:, :, :, DATA_OFF : DATA_OFF + W] = x[:, pe_chs].transpose(1, 2, 0, 3)

        # tmat blocks: per PE channel [hin, dx, hout] -> [(hin dx), hout]
        blocks = []
        for ci, gch in enumerate(pe_chs):
            k = KS_PE[ci]
            Tc = Ts[k][gch % GROUP_SIZE]  # [dx, hin, hout]
            blocks.append(
                np.ascontiguousarray(Tc.transpose(1, 0, 2)).reshape(k * H, H)
            )
        tml = np.concatenate(blocks, axis=0)
        assert tml.shape[0] == N_TMAT * H
        m = {"xp": xp, "tmat": tml.astype(BF16)}

        if N3:
            # staged x (DVE): [N3, 128, XDF]; partition = img*4 + hblock,
            # free = halo'd 30x114 patch
            xd = np.zeros((N3, 128, XDF), BF16)
            xdv = xd[:, :, : RH * RWP].reshape(N3, N_IMGS, 4, RH, RWP)
            for di, gch in enumerate(dve_chs):
                pad_img = np.zeros((N_IMGS, H + 2, RWP), BF16)
                pad_img[:, 1 : H + 1, 1 : W + 1] = x[:, gch]
                for hb in range(4):
                    xdv[di, :, hb] = pad_img[:, HB * hb : HB * hb + RH, :]
            # tap weights broadcast across partitions: [128, N3*9]
            wd = np.zeros((N3, 9), np.float32)
            for di, gch in enumerate(dve_chs):
                wd[di] = ws[3][gch % GROUP_SIZE, 0].reshape(9)
            m["xd"] = xd
            m["wdve"] = np.ascontiguousarray(
                np.broadcast_to(wd.reshape(1, N3 * 9), (128, N3 * 9))
            )
        in_maps.append(m)
    return in_maps


def _gather(results):
    out = np.empty((N_IMGS, GROUP_SIZE * len(GROUP_KS), H, W), np.float32)
    for core in range(N_CORES):
        chs = _core_channels(core)
        pe_chs = [chs[i] for i in PE_POS]
        y = np.asarray(results[core]["y"]).astype(np.float32)
        y = y.reshape(N_PE, H, N_IMGS, W)
        out[:, pe_chs] = y.transpose(2, 0, 1, 3)
        if N3:
            dve_chs = [chs[i] for i in DVE_POS]
            y2 = np.asarray(results[core]["y2"]).astype(np.float32)
            y2 = y2.reshape(N3, N_IMGS, 4, HB, W)
            for di, gch in enumerate(dve_chs):
                out[:, gch] = y2[di].reshape(N_IMGS, H, W)
    return out


def run(x, w3, w5, w7, **spmd_kwargs):
    """Full run; returns (output, BassKernelResults) for profiling access."""
    nc = _get_bass()
    in_maps = _prepare_in_maps(x, w3, w5, w7)
    br = run_bass_kernel_spmd(nc, in_maps, core_ids=list(range(N_CORES)), **spmd_kwargs)
    return _gather(br.results), br


def kernel(x, w3, w5, w7):
    out, _ = run(x, w3, w5, w7)
    return out

